# revision 29
# baseline (speedup 1.0000x reference)
"""Multi-head attention (S=2048, B=2, D=1024, H=16) on 8 Trainium2 cores.

Sharding: tensor-parallel over heads (4 groups of 4 heads) x data-parallel
over batch (2). Core r handles batch r//4, heads [4*(r%4), 4*(r%4)+4).

Structure (v2): the scalar engine's 128 Exp activations (~1.35us each) are
the roofline; everything else is arranged around keeping that stream dense:
  - minimal prefix (one K-proj quarter + one Q-proj quarter), all other
    projections and the output projection woven into attention steps;
  - Q bias folded in as a 9th K=1 matmul (ones row x bias row); K bias
    dropped entirely (softmax is invariant to per-query constants), so the
    scalar engine runs Exp only;
  - inputs arrive as 1MB quarter-DMAs in consumption order;
  - ReduceScatter split into 7 chunks (6x256 + 512 rows) so mid-stream RS
    ops (~16us) hide under compute and only the last (~23us) is exposed.

All matmul operands bf16, fp32 PSUM accumulation. Softmax denominators come
free from a ones-column appended to V. V's bias and the output bias are
folded out algebraically and added on the host.
"""
import sys

sys.path.insert(0, "/opt/trn_rl_repo")

import numpy as np
import ml_dtypes
import concourse.bacc as bacc
import concourse.mybir as mybir
from concourse import tile
from concourse.bass_utils import run_bass_kernel_spmd

dt = mybir.dt
AF = mybir.ActivationFunctionType
BF16 = ml_dtypes.bfloat16

S, B, D = 2048, 2, 1024
H, DK = 16, 64
NCORES = 8
HC = 4                 # heads per core
CH = HC * DK           # 256 local channels per core
SCALE = np.float32(1.0 / np.sqrt(DK))
GROUPS = [[0, 1, 2, 3], [4, 5, 6, 7]]

NKD = D // 128         # 8 contraction tiles for projections
NTK = S // 128         # 16 key tiles
NB = 4                 # attention blocks of 512 q tokens
PV_LAG = 5

# wpack column offsets: [wk 2048 | ones 4 | wq 2048 | wv 2048 | wo 2048]
WK0, ON0, WQ0, WV0, WO0 = 0, 2048, 2052, 4100, 6148
WPCOLS = 8196

# RS chunks over output rows: block-aligned 4 x 512 (subtiles 128 rows);
# one RS per attention block keeps the serial CC stream far under-subscribed
# so only the final block's RS is exposed at the tail
CHUNK_SUBS = [[0, 1, 2, 3], [4, 5, 6, 7], [8, 9, 10, 11], [12, 13, 14, 15]]
NCH = len(CHUNK_SUBS)
CHROW0 = [c[0] * 128 for c in CHUNK_SUBS]
CHROWS = [len(c) * 128 for c in CHUNK_SUBS]


def build_nc():
    f32, bf16 = dt.float32, dt.bfloat16
    nc = bacc.Bacc("TRN2", target_bir_lowering=False, debug=False,
                   num_devices=NCORES)

    # inputs: [128, 16384] with layout (p, quarter, k, t) per tensor
    xq = nc.dram_tensor("xq_t", [128, 16384], bf16, kind="ExternalInput").ap()
    xk = nc.dram_tensor("xk_t", [128, 16384], bf16, kind="ExternalInput").ap()
    xv = nc.dram_tensor("xv_t", [128, 16384], bf16, kind="ExternalInput").ap()
    wpk = nc.dram_tensor("wpack", [128, WPCOLS], bf16,
                         kind="ExternalInput").ap()
    # [1, 768]: [bq j0 128 | bq j1 128 | ones 512]
    bqo = nc.dram_tensor("bqo", [1, 768], bf16, kind="ExternalInput").ap()
    out_ext = nc.dram_tensor("out_rs", [S // 4, D], bf16,
                             kind="ExternalOutput").ap()

    with tile.TileContext(nc) as tc:
        with tc.tile_pool(name="const", bufs=1) as cp, \
             tc.tile_pool(name="stream", bufs=1) as sp, \
             tc.tile_pool(name="psum", bufs=1, space="PSUM") as pp, \
             tc.tile_pool(name="dram", bufs=1, space="DRAM") as dp:

            # ---- CC warmup: absorb the collective stream's init cost ----
            cc_warm_in = dp.tile([16, D], bf16, tag="ccwi", name="cc_warm_in")
            cc_warm_out = dp.tile([4, D], bf16, tag="ccwo",
                                  name="cc_warm_out")
            nc.gpsimd.collective_compute(
                "ReduceScatter", mybir.AluOpType.add,
                replica_groups=GROUPS,
                ins=[cc_warm_in[:]], outs=[cc_warm_out[:]])

            # ---- weights / bias ----
            bqo_sb = cp.tile([1, 768], bf16, tag="bqo", name="bqo_sb")
            nc.scalar.dma_start(bqo_sb[:], bqo[:])
            wpack_sb = cp.tile([128, WPCOLS], bf16, tag="wpack",
                               name="wpack_sb")
            nc.scalar.dma_start(wpack_sb[:, WK0:WQ0], wpk[:, WK0:WQ0])
            nc.scalar.dma_start(wpack_sb[:, WQ0:WV0], wpk[:, WQ0:WV0])
            # wv arrives via gpsimd (below); wo is deferred into the weave
            wk_sb = [wpack_sb[:, WK0 + k * CH:WK0 + (k + 1) * CH]
                     for k in range(NKD)]
            wq_sb = [wpack_sb[:, WQ0 + k * CH:WQ0 + (k + 1) * CH]
                     for k in range(NKD)]
            wv_sb = [wpack_sb[:, WV0 + k * CH:WV0 + (k + 1) * CH]
                     for k in range(NKD)]
            wo_sb = [wpack_sb[:, WO0 + k * D:WO0 + (k + 1) * D]
                     for k in range(2)]
            ones_sb = wpack_sb[:, ON0:ON0 + 4]

            # exp table preload so the first real Exp doesn't pay ~2.7us
            pre_sb = cp.tile([1, 16], f32, tag="pre", name="pre_sb")
            nc.vector.memset(pre_sb[:], 0.0)
            pre_o = cp.tile([1, 16], f32, tag="preo", name="preo_sb")
            nc.scalar.activation(pre_o[:], pre_sb[:], AF.Exp)

            # ---- input quarters (1MB DMAs, consumption order) ----
            # quarter 0 of xk/xq is split in halves spread across rings so
            # the prefix's first bytes land as early as possible
            xkq, xqq, xvq = {}, {}, {}

            def load_q(dst_map, tagc, src, qi, eng, bufs=3):
                t_ = sp.tile([128, 4096], bf16, tag=f"x{tagc}",
                             bufs=bufs, name=f"x{tagc}{qi}")
                eng.dma_start(t_[:], src[:, qi * 4096:(qi + 1) * 4096])
                dst_map[qi] = t_

            def load_h(dst_map, tagc, src, half, eng):
                t_ = sp.tile([128, 2048], bf16, tag=f"x{tagc}h{half}",
                             bufs=1, name=f"x{tagc}0{half}")
                eng.dma_start(t_[:], src[:, half * 2048:(half + 1) * 2048])
                dst_map[(0, half)] = t_

            # the first-exp critical bytes (wk+wq / xk0 / xq0 ~ 1MB per
            # ring) are spread evenly across the three rings; everything
            # else follows in consumption order
            load_h(xkq, "k", xk, 0, nc.sync)     # sync:   xk0a first
            load_h(xqq, "q", xq, 1, nc.sync)     #         xq0b second
            load_q(xvq, "v", xv, 0, nc.sync, bufs=4)
            load_q(xkq, "k", xk, 1, nc.sync)
            load_q(xvq, "v", xv, 1, nc.sync, bufs=4)
            load_q(xkq, "k", xk, 2, nc.sync)
            load_q(xvq, "v", xv, 2, nc.sync, bufs=4)
            load_q(xkq, "k", xk, 3, nc.sync)
            load_q(xvq, "v", xv, 3, nc.sync, bufs=4)
            load_h(xkq, "k", xk, 1, nc.gpsimd)   # gpsimd: xk0b
            load_h(xqq, "q", xq, 0, nc.gpsimd)   #         xq0a
            nc.gpsimd.dma_start(wpack_sb[:, WV0:WO0], wpk[:, WV0:WO0])
            load_q(xqq, "q", xq, 1, nc.gpsimd)
            nc.gpsimd.dma_start(wpack_sb[:, WO0:WPCOLS], wpk[:, WO0:WPCOLS])
            load_q(xqq, "q", xq, 2, nc.gpsimd)
            load_q(xqq, "q", xq, 3, nc.gpsimd)

            def xslice(m, qi, k):
                if qi == 0 and (0, 0) in m:
                    return m[(0, k // 4)][:, (k % 4) * 512:
                                          (k % 4) * 512 + 512]
                return m[qi][:, k * 512:(k + 1) * 512]

            # ---- persistent activations ----
            qc = [cp.tile([128, S], bf16, tag=f"qc{j}", name=f"qc{j}")
                  for j in range(2)]
            kc = [cp.tile([128, S], bf16, tag=f"kc{j}", name=f"kc{j}")
                  for j in range(2)]
            vt = [cp.tile([128, HC * (DK + 1)], bf16, tag=f"vt{t}",
                          name=f"vt{t}") for t in range(NTK)]
            ctx = [cp.tile([128, S], bf16, tag=f"ctx{j}", name=f"ctx{j}")
                   for j in range(2)]
            for t in range(NTK):
                vt_view = vt[t][:].rearrange("p (h c) -> p h c", h=HC)
                nc.vector.tensor_copy(vt_view[:, :, DK:DK + 1],
                                      ones_sb.unsqueeze(2))

            # ---- projection emitters as generators: each yields an
            # approximate PE cost (ns) after every matmul so the weave
            # pump can spread them one matmul at a time ----
            def kproj_gen(th, j):
                ps = pp.tile([128, 512], f32, tag="pj", bufs=2,
                             name=f"kp{th}_{j}")
                for k in range(NKD):
                    nc.tensor.matmul(
                        ps[:], wk_sb[k][:, j * 128:(j + 1) * 128],
                        xslice(xkq, th, k),
                        start=(k == 0), stop=(k == NKD - 1))
                    yield 265
                nc.vector.tensor_copy(kc[j][:, th * 512:(th + 1) * 512],
                                      ps[:])

            def qproj_gen(th, j):
                ps = pp.tile([128, 512], f32, tag="pj", bufs=2,
                             name=f"qp{th}_{j}")
                for k in range(NKD):
                    nc.tensor.matmul(
                        ps[:], wq_sb[k][:, j * 128:(j + 1) * 128],
                        xslice(xqq, th, k),
                        start=(k == 0), stop=False)
                    yield 265
                # 9th matmul adds bq via ones row (K=1)
                nc.tensor.matmul(
                    ps[:], bqo_sb[0:1, j * 128:(j + 1) * 128],
                    bqo_sb[0:1, 256:768],
                    start=False, stop=True)
                yield 265
                nc.vector.tensor_copy(qc[j][:, th * 512:(th + 1) * 512],
                                      ps[:])

            def vproj_gen(t, p):
                # V projection for key tile t, head pair p (128 channels)
                ps = pp.tile([128, 512], f32, tag="pj", bufs=2,
                             name=f"vp{t}_{p}")
                for k in range(NKD):
                    nc.tensor.matmul(
                        ps[:, 0:128],
                        xslice(xvq, t // 4, k)[:, (t % 4) * 128:
                                               (t % 4) * 128 + 128],
                        wv_sb[k][:, p * 128:(p + 1) * 128],
                        start=(k == 0), stop=(k == NKD - 1))
                    if k % 2 == 1:
                        yield 150
                dst = vt[t][:, p * 130:(p + 1) * 130].rearrange(
                    "p (h c) -> p h c", h=2)
                src = ps[:, 0:128].rearrange("p (h c) -> p h c", h=2)
                nc.vector.tensor_copy(dst[:, :, 0:DK], src)

            # ---- collective buffers ----
            cc_ins = [dp.tile([CHROWS[c], D], bf16, tag=f"ccin{c}",
                              name=f"cc_in{c}") for c in range(NCH)]
            cc_outs = [dp.tile([CHROWS[c] // 4, D], bf16, tag=f"ccout{c}",
                               name=f"cc_out{c}") for c in range(NCH)]

            def chunk_rs(c):
                nc.gpsimd.collective_compute(
                    "ReduceScatter", mybir.AluOpType.add,
                    replica_groups=GROUPS,
                    ins=[cc_ins[c][:]], outs=[cc_outs[c][:]])

            def outproj_gen(sub, tail=False):
                t0 = sub * 128
                osb = sp.tile([128, D], bf16, tag="ot", bufs=4,
                              name=f"ot{sub}")
                if tail:
                    # scores are done; reuse the wide s1 psum ring so the
                    # four matmuls pipeline with a single evacuation copy
                    po = pp.tile([128, 1024], f32, tag="s1", bufs=2,
                                 name=f"pot{sub}")
                    for e in range(2):
                        for dv in range(2):
                            nc.tensor.matmul(
                                po[:, e * 512:(e + 1) * 512],
                                ctx[dv][:, t0:t0 + 128],
                                wo_sb[dv][:, e * 512:(e + 1) * 512],
                                start=(dv == 0), stop=(dv == 1))
                    nc.vector.tensor_copy(osb[:], po[:])
                else:
                    for e in range(2):
                        po = pp.tile([128, 512], f32, tag="pj", bufs=2,
                                     name=f"po{sub}_{e}")
                        for dv in range(2):
                            nc.tensor.matmul(
                                po[:], ctx[dv][:, t0:t0 + 128],
                                wo_sb[dv][:, e * 512:(e + 1) * 512],
                                start=(dv == 0), stop=(dv == 1))
                            yield 265
                        nc.vector.tensor_copy(osb[:, e * 512:(e + 1) * 512],
                                              po[:])
                c = next(i for i, subs in enumerate(CHUNK_SUBS)
                         if sub in subs)
                r0 = t0 - CHROW0[c]
                nc.sync.dma_start(cc_ins[c][r0:r0 + 128, :], osb[:])
                if sub == CHUNK_SUBS[c][-1]:
                    chunk_rs(c)

            # ---- normalize (per pair, per block) ----
            def emit_normalize(bi, p, cx):
                tq0 = bi * 512
                cxs = []
                for h in range(2):
                    c_ = sp.tile([65, 512], f32, tag="cxs", bufs=4,
                                 name=f"cxs{p}_{h}")
                    nc.vector.tensor_copy(c_[:], cx[h][:])
                    cxs.append(c_)
                for h in range(2):
                    den = sp.tile([1, 512], f32, tag="den", bufs=2,
                                  name=f"den{p}_{h}")
                    nc.vector.tensor_copy(den[:], cxs[h][64:65, :])
                    rc = sp.tile([1, 512], f32, tag="rc", bufs=2,
                                 name=f"rc{p}_{h}")
                    nc.vector.reciprocal_approx_fast(rc[:], den[:])
                    bc = sp.tile([64, 512], f32, tag="bc", bufs=4,
                                 name=f"bc{p}_{h}")
                    nc.gpsimd.partition_broadcast(bc[:], rc[:])
                    nc.vector.tensor_mul(
                        ctx[p][h * 64:(h + 1) * 64, tq0:tq0 + 512],
                        cxs[h][0:64, :], bc[:])

            # ---- deferred-PV machinery ----
            pvq = []

            def pop_pv():
                bi_, p_, tk_, etf_, cx_ = pvq.pop(0)
                force(("vp", tk_, p_))
                for h in range(2):
                    hl = p_ * 2 + h
                    nc.tensor.matmul(
                        cx_[h][:],
                        vt[tk_][:, hl * 65:(hl + 1) * 65],
                        etf_[:, h * 512:(h + 1) * 512],
                        start=(tk_ == 0), stop=(tk_ == NTK - 1))
                if tk_ == NTK - 1:
                    emit_normalize(bi_, p_, cx_)

            # ---- weave task registry: ordered tasks advanced one matmul
            # at a time by a per-step budget pump; data dependencies are
            # guaranteed by force() calls at the consuming step ----
            tasks = {}          # key -> generator (live) | None (done)
            order = []          # keys in FIFO pump order

            def add_task(start, key, genf):
                tasks[key] = ("new", genf)
                order.append((start, key))

            def _ensure(key):
                st = tasks.get(key)
                if st is None or st == "done":
                    return None
                if isinstance(st, tuple) and st[0] == "new":
                    g = st[1]()
                    tasks[key] = g
                    return g
                return st

            def force(key):
                g = _ensure(key)
                if g is None:
                    return
                for _ in g:
                    pass
                tasks[key] = "done"

            def pump(step, budget):
                for start, key in order:
                    if budget <= 0:
                        return
                    if start > step:
                        return
                    g = _ensure(key)
                    if g is None:
                        continue
                    while budget > 0:
                        try:
                            budget -= next(g)
                        except StopIteration:
                            tasks[key] = "done"
                            break

            def dma_gen(fn):
                fn()
                return
                yield

            add_task(0, ("kq", 0, 0), lambda: kproj_gen(0, 0))
            add_task(0, ("qp", 0, 0), lambda: qproj_gen(0, 0))
            add_task(1, ("kq", 1, 0), lambda: kproj_gen(1, 0))
            add_task(3, ("kq", 2, 0), lambda: kproj_gen(2, 0))
            add_task(5, ("kq", 3, 0), lambda: kproj_gen(3, 0))
            for t in range(16):
                add_task(t + 1, ("vp", t, 0), lambda t=t: vproj_gen(t, 0))
            add_task(10, ("qp", 0, 1), lambda: qproj_gen(0, 1))
            add_task(11, ("kq", 0, 1), lambda: kproj_gen(0, 1))
            add_task(13, ("kq", 1, 1), lambda: kproj_gen(1, 1))
            for t in range(16):
                add_task(14 + t, ("vp", t, 1), lambda t=t: vproj_gen(t, 1))
            add_task(16, ("kq", 2, 1), lambda: kproj_gen(2, 1))
            add_task(19, ("kq", 3, 1), lambda: kproj_gen(3, 1))
            add_task(24, ("qp", 1, 0), lambda: qproj_gen(1, 0))
            add_task(27, ("qp", 1, 1), lambda: qproj_gen(1, 1))
            add_task(54, ("qp", 2, 0), lambda: qproj_gen(2, 0))
            add_task(58, ("qp", 2, 1), lambda: qproj_gen(2, 1))
            add_task(86, ("qp", 3, 0), lambda: qproj_gen(3, 0))
            add_task(90, ("qp", 3, 1), lambda: qproj_gen(3, 1))
            op_due = {}
            for bi in range(1, NB):
                for i, sub in enumerate(range((bi - 1) * 4, bi * 4)):
                    add_task(bi * 32 + 6 + 2 * i, ("op", sub),
                             lambda sub=sub: outproj_gen(sub))
                    op_due[bi * 32 + 8 + 2 * i] = sub
            order.sort(key=lambda x: x[0])

            # ---- prefix compute ----
            force(("kq", 0, 0))
            force(("qp", 0, 0))

            # ---- attention stream ----
            for bi in range(NB):
                tq0 = bi * 512
                for p in range(2):
                    cx_cur = [pp.tile([65, 512], f32, tag="cx", bufs=2,
                                      name=f"cx{p}_{h}") for h in range(2)]
                    for tk in range(NTK):
                        s = bi * 32 + p * 16 + tk
                        # hard data deadlines for this step's scores
                        force(("kq", tk // 4, p))
                        if tk == 0:
                            force(("qp", bi, p))
                        s1 = pp.tile([128, 1024], f32, tag="s1", bufs=2,
                                     name=f"s1_{tk}")
                        etf = sp.tile([128, 1024], bf16, tag="et", bufs=8,
                                      name=f"et{tk}")
                        for h in range(2):
                            r0 = h * 64
                            nc.tensor.matmul(
                                s1[:, h * 512:(h + 1) * 512],
                                kc[p][r0:r0 + 64, tk * 128:(tk + 1) * 128],
                                qc[p][r0:r0 + 64, tq0:tq0 + 512],
                                start=True, stop=True)
                        nc.scalar.activation(etf[:], s1[:], AF.Exp)
                        pvq.append((bi, p, tk, etf, cx_cur))
                        while len(pvq) > PV_LAG:
                            pop_pv()
                        if s in op_due:
                            force(("op", op_due[s]))
                        pump(s, 400)
            while pvq:
                pop_pv()
            # drain any unfinished woven work, then the last block's
            # outproj + final chunk RS
            for start, key in order:
                force(key)
            for sub in range(12, 16):
                force_gen = outproj_gen(sub, tail=True)
                for _ in force_gen:
                    pass

            # final stores, scheduled at the very end
            with tc.tile_wait_until(10):
                for c in range(NCH):
                    o0 = CHROW0[c] // 4
                    eng = nc.sync if c % 2 == 0 else nc.gpsimd
                    eng.dma_start(out_ext[o0:o0 + CHROWS[c] // 4, :],
                                  cc_outs[c][:])

    nc.finalize()
    return nc


_NC = None


def _get_nc():
    global _NC
    if _NC is None:
        _NC = build_nc()
    return _NC


def _pack_x(xb):
    # [1024, 2048] (d, s) -> [128, 16384] layout (p, quarter, k, t)
    return np.ascontiguousarray(
        xb.reshape(NKD, 128, 4, 512).transpose(1, 2, 0, 3).reshape(
            128, 16384)).astype(BF16)


def make_in_maps(q, k, v, Wq, bq, Wk, bk, Wv, bv, Wo, bo):
    """Shard + precondition full inputs into per-core input maps."""
    xq_b = [_pack_x(np.asarray(q[:, b, :].T, dtype=np.float32))
            for b in range(B)]
    xk_b = [_pack_x(np.asarray(k[:, b, :].T, dtype=np.float32))
            for b in range(B)]
    xv_b = [_pack_x(np.asarray(v[:, b, :].T, dtype=np.float32))
            for b in range(B)]
    in_maps = []
    for r in range(NCORES):
        b = r // 4
        g = r % 4
        ch = slice(g * CH, (g + 1) * CH)

        def pack_dk(wt, width):
            nk = wt.shape[0] // 128
            return wt.reshape(nk, 128, width).transpose(1, 0, 2).reshape(
                128, nk * width)

        wk_t = np.ascontiguousarray(Wk[ch, :].T).astype(BF16)
        wq_t = np.ascontiguousarray((Wq[ch, :] * SCALE).T).astype(BF16)
        wv_t = np.ascontiguousarray(Wv[ch, :].T).astype(BF16)
        wo_t = np.ascontiguousarray(Wo[:, ch].T).astype(BF16)
        wpack = np.concatenate([
            pack_dk(wk_t, CH), np.ones((128, 4), dtype=BF16),
            pack_dk(wq_t, CH), pack_dk(wv_t, CH),
            pack_dk(wo_t, D)], axis=1)
        bqo = np.concatenate([
            (np.asarray(bq[ch], dtype=np.float32) * SCALE).astype(BF16),
            np.ones((512,), dtype=BF16)]).reshape(1, 768)
        in_maps.append({
            "xq_t": xq_b[b], "xk_t": xk_b[b], "xv_t": xv_b[b],
            "wpack": np.ascontiguousarray(wpack),
            "bqo": np.ascontiguousarray(bqo),
        })
    return in_maps


def assemble(results, Wo, bv, bo):
    """Gather per-core RS slices into the full [S, B, D] output."""
    out = np.empty((S, B, D), dtype=np.float32)
    for r in range(NCORES):
        b = r // 4
        j = r % 4
        for c in range(NCH):
            rows = CHROWS[c] // 4
            g0 = CHROW0[c] + j * rows        # global token rows
            o0 = CHROW0[c] // 4              # rows within out_rs
            out[g0:g0 + rows, b, :] = \
                results[r]["out_rs"][o0:o0 + rows].astype(np.float32)
    out += (bo + Wo @ bv).astype(np.float32)
    return out


def run_sharded(inputs, trace=False):
    nc = _get_nc()
    in_maps = make_in_maps(**inputs)
    res = run_bass_kernel_spmd(nc, in_maps, list(range(NCORES)), trace=trace)
    full = assemble(res.results, np.asarray(inputs["Wo"], dtype=np.float32),
                    np.asarray(inputs["bv"], dtype=np.float32),
                    np.asarray(inputs["bo"], dtype=np.float32))
    return full, res


def kernel(**inputs) -> np.ndarray:
    inputs = {k_: np.asarray(v_, dtype=np.float32)
              for k_, v_ in inputs.items()}
    full, _ = run_sharded(inputs)
    return full
